# revision 11
# baseline (speedup 1.0000x reference)
"""GNN message-passing kernel for Trainium2 (8 NeuronCores, SPMD).

Strategy:
  - Sort edges by dst on host; each core owns a contiguous 1280-node range
    (10 blocks of 128 nodes) and all edges targeting it. No collectives.
  - Node-level K/V tables (xx@W_k+b etc.) computed replicated on every core,
    stored in DRAM, gathered per-edge via indirect DMA (src is uniform).
  - Q is never gathered: per 128-node block, Q_blk is computed on the fly and
    expanded to edges with a one-hot matmul (lhsT=OH[node,edge]).
  - The segment-sum (scatter-add by dst) is a one-hot matmul per tile
    accumulating into a per-block PSUM bank.
  - All biases are folded into the tables / K=1 bias-row matmuls on host.
"""

import numpy as np

import concourse.bass as bass
import concourse.tile as tile
from concourse import bacc, mybir
from concourse.bass_utils import run_bass_kernel_spmd
from concourse.masks import make_identity

# problem constants (hardcoded per contract)
N = 10000
E = 160000
W = 256
HID = 512
EW = 128
HD = 64
NH = 8
NCORES = 8
P = 128
BLOCKS = 10            # node blocks per core
NLOC = BLOCKS * P      # 1280 local node slots per core
NT_A = (N + P - 1) // P  # 79 node tiles for table build

F32 = mybir.dt.float32
I32 = mybir.dt.int32


def _host_prep(inputs):
    """Sort edges by dst, shard by owner core, pad blocks to uniform tiles."""
    x = np.ascontiguousarray(np.asarray(inputs["x"], np.float32))
    ei = np.asarray(inputs["edge_index"], np.int64)
    attr = np.asarray(inputs["edge_attr"], np.int64)
    eemb = np.ascontiguousarray(np.asarray(inputs["edge_embed"], np.float32))

    src, dst = ei[0], ei[1]
    order = np.argsort(dst, kind="stable")
    src_s = src[order].astype(np.int32)
    dst_s = dst[order].astype(np.int32)
    attr_s = attr[order].astype(np.int32)
    eemb_s = eemb[order]

    # per global 128-node block: edge ranges
    n_gblocks = NCORES * BLOCKS  # 80 slots (last ones may be empty)
    block_of_edge = dst_s // P
    starts = np.searchsorted(block_of_edge, np.arange(n_gblocks))
    ends = np.searchsorted(block_of_edge, np.arange(n_gblocks) + 1)
    counts = ends - starts
    t_max = max(1, int(np.max((counts + P - 1) // P)))
    e_pad = BLOCKS * t_max * P
    nt = BLOCKS * t_max  # edge tiles per core

    # weight prep
    init0_e = np.asarray(inputs["init0_e"], np.float32)
    init0 = np.asarray(inputs["init0"], np.float32)
    xw = np.exp(init0_e) / np.sqrt(np.sum(np.exp(init0_e)))
    s_e = float(np.exp(init0[-2]))
    s_v = float(np.exp(init0[-1]))
    att_scale = float(init0[0]) / np.sqrt(HD)
    att_bias = float(init0[1])

    emb12 = np.zeros((12, EW), np.float32)
    for t in range(4):
        tab = np.asarray(inputs[f"emb{t}"], np.float32)
        emb12[3 * t : 3 * t + 3] = tab[0:3] * xw[t]

    wr0 = np.asarray(inputs["W_r0"], np.float32) * s_e
    wr1 = np.asarray(inputs["W_r1"], np.float32) * s_e
    wr2 = np.asarray(inputs["W_r2"], np.float32) * s_v
    bq = (np.asarray(inputs["b_q"]) + np.asarray(inputs["b_r0"]) * s_e).astype(np.float32)
    bk = (np.asarray(inputs["b_k"]) + np.asarray(inputs["b_r1"]) * s_e).astype(np.float32)
    bv = (np.asarray(inputs["b_v"]) + np.asarray(inputs["b_r2"]) * s_v).astype(np.float32)

    weights = {
        "W_pre": np.ascontiguousarray(inputs["W_pre"], np.float32),
        "b_pre": np.asarray(inputs["b_pre"], np.float32).reshape(1, W),
        "W_q": np.ascontiguousarray(inputs["W_q"], np.float32),
        "W_k": np.ascontiguousarray(inputs["W_k"], np.float32),
        "W_v": np.ascontiguousarray(inputs["W_v"], np.float32),
        "W_msg0": np.ascontiguousarray(inputs["W_msg0"], np.float32),
        "b_msg0": np.asarray(inputs["b_msg0"], np.float32).reshape(1, HID),
        "W_post": np.ascontiguousarray(inputs["W_post"], np.float32),
        "b_post": np.asarray(inputs["b_post"], np.float32).reshape(1, W),
        "W_r0s": wr0, "W_r1s": wr1, "W_r2s": wr2,
        "bq_tot": bq.reshape(1, HID), "bk_tot": bk.reshape(1, HID),
        "bv_tot": bv.reshape(1, HID),
        "emb12": emb12,
        "iota_row": np.arange(P, dtype=np.float32).reshape(1, P),
        "x": x,
    }

    # per-core shard arrays
    shards = []
    for c in range(NCORES):
        ee_sh = np.zeros((e_pad, EW), np.float32)
        oh12 = np.zeros((12, e_pad), np.float32)
        srcw = np.zeros((P, nt), np.int32)
        dstrelf = np.full((P, nt), 999.0, np.float32)
        for b in range(BLOCKS):
            g = c * BLOCKS + b
            if g >= n_gblocks:
                continue
            s0, e0 = int(starts[g]), int(ends[g])
            cnt = e0 - s0
            if cnt == 0:
                continue
            base = b * t_max * P
            ee_sh[base : base + cnt] = eemb_s[s0:e0]
            a = attr_s[s0:e0]  # [cnt, 4] values in 0..2
            cols = np.arange(cnt)
            for t in range(4):
                oh12[3 * t + 0, base + cols] = (a[:, t] == 0)
                oh12[3 * t + 1, base + cols] = (a[:, t] == 1)
                oh12[3 * t + 2, base + cols] = (a[:, t] == 2)
            # wrapped [128, tile] layouts
            for ti in range((cnt + P - 1) // P):
                lo = ti * P
                hi = min(lo + P, cnt)
                col = b * t_max + ti
                srcw[0 : hi - lo, col] = src_s[s0 + lo : s0 + hi]
                dstrelf[0 : hi - lo, col] = (dst_s[s0 + lo : s0 + hi] - g * P).astype(np.float32)
        xg = np.zeros((NLOC, W), np.float32)
        n0 = c * NLOC
        n1 = min(n0 + NLOC, N)
        xg[0 : n1 - n0] = x[n0:n1]
        shards.append({"ee_sh": ee_sh, "oh12": oh12, "srcw": srcw,
                       "dstrelf": dstrelf, "xg": xg})
    return weights, shards, t_max, e_pad, nt, att_scale, att_bias


def _build_program(t_max, e_pad, nt, att_scale, att_bias):
    global att_scale_g, att_bias_g
    att_scale_g, att_bias_g = att_scale, att_bias
    nc = bacc.Bacc("TRN2", target_bir_lowering=False, debug=False,
                   num_devices=NCORES)

    # DRAM tensors
    d = {}
    d["x"] = nc.dram_tensor("x", [N, W], F32, kind="ExternalInput").ap()
    d["xg"] = nc.dram_tensor("xg", [NLOC, W], F32, kind="ExternalInput").ap()
    d["ee_sh"] = nc.dram_tensor("ee_sh", [e_pad, EW], F32, kind="ExternalInput").ap()
    d["oh12"] = nc.dram_tensor("oh12", [12, e_pad], F32, kind="ExternalInput").ap()
    d["srcw"] = nc.dram_tensor("srcw", [P, nt], I32, kind="ExternalInput").ap()
    d["dstrelf"] = nc.dram_tensor("dstrelf", [P, nt], F32, kind="ExternalInput").ap()
    for nm, shp in [("W_pre", [W, W]), ("b_pre", [1, W]),
                    ("W_q", [W, HID]), ("W_k", [W, HID]), ("W_v", [W, HID]),
                    ("W_msg0", [W, HID]), ("b_msg0", [1, HID]),
                    ("W_post", [HID, W]), ("b_post", [1, W]),
                    ("W_r0s", [EW, HID]), ("W_r1s", [EW, HID]), ("W_r2s", [EW, HID]),
                    ("bq_tot", [1, HID]), ("bk_tot", [1, HID]), ("bv_tot", [1, HID]),
                    ("emb12", [12, EW]), ("iota_row", [1, P])]:
        d[nm] = nc.dram_tensor(nm, shp, F32, kind="ExternalInput").ap()
    ktab = nc.dram_tensor("ktab", [N, HID], F32, kind="Internal").ap()
    vtab = nc.dram_tensor("vtab", [N, HID], F32, kind="Internal").ap()
    out_d = nc.dram_tensor("out", [NLOC, W], F32, kind="ExternalOutput").ap()

    with tile.TileContext(nc) as tc:
        _emit(tc, d, ktab, vtab, out_d, t_max)

    nc.compile()
    return nc


def _emit(tc, d, ktab, vtab, out_d, t_max):
    from contextlib import ExitStack

    nc = tc.nc
    ctx = ExitStack()
    wp = ctx.enter_context(tc.tile_pool(name="weights", bufs=1))

    # persistent small tensors
    ident = wp.tile([P, P], F32, tag="ident")
    make_identity(nc, ident[:])
    ones1 = wp.tile([1, P], F32, tag="ones1")
    nc.gpsimd.memset(ones1[:], 1.0)
    iota_bc = wp.tile([P, P], F32, tag="iota_bc")
    nc.sync.dma_start(iota_bc[:], d["iota_row"][0:1, :].to_broadcast((P, P)))
    zero1 = wp.tile([P, 1], F32, tag="zero1")
    nc.vector.memset(zero1[:], 0.0)
    eps1 = wp.tile([P, 1], F32, tag="eps1")
    nc.vector.memset(eps1[:], 1e-5)
    attb1 = wp.tile([P, 1], F32, tag="attb1")
    nc.vector.memset(attb1[:], att_bias_g)

    def load_w(name, parts, cols):
        ts_ = []
        for i in range(parts):
            t = wp.tile([P, cols], F32, tag=f"{name}{i}")
            nc.sync.dma_start(t[:], d[name][i * P : (i + 1) * P, :])
            ts_.append(t)
        return ts_

    w_pre = load_w("W_pre", 2, W)
    w_q = load_w("W_q", 2, HID)
    w_k = load_w("W_k", 2, HID)
    w_v = load_w("W_v", 2, HID)
    w_msg0 = load_w("W_msg0", 2, HID)
    w_post = load_w("W_post", 4, W)
    w_r = []
    for i in range(3):
        t = wp.tile([EW, HID], F32, tag=f"wr{i}")
        nc.sync.dma_start(t[:], d[f"W_r{i}s"][:, :])
        w_r.append(t)
    emb12 = wp.tile([12, EW], F32, tag="emb12")
    nc.sync.dma_start(emb12[:], d["emb12"][:, :])
    small_rows = {}
    for nm, cols in [("b_pre", W), ("bq_tot", HID), ("bk_tot", HID),
                     ("bv_tot", HID), ("b_msg0", HID), ("b_post", W)]:
        t = wp.tile([1, cols], F32, tag=nm)
        nc.sync.dma_start(t[:], d[nm][:, :])
        small_rows[nm] = t
    srcw = wp.tile([P, d["srcw"].shape[1]], I32, tag="srcw")
    nc.sync.dma_start(srcw[:], d["srcw"][:, :])
    dstrelf = wp.tile([P, d["dstrelf"].shape[1]], F32, tag="dstrelf")
    nc.sync.dma_start(dstrelf[:], d["dstrelf"][:, :])

    xxT_loc0 = wp.tile([P, NLOC], F32, tag="xxT0")
    xxT_loc1 = wp.tile([P, NLOC], F32, tag="xxT1")
    xxT_loc = [xxT_loc0, xxT_loc1]

    # pools
    sb = ctx.enter_context(tc.tile_pool(name="sb", bufs=3))
    sb2 = ctx.enter_context(tc.tile_pool(name="sb2", bufs=2))
    pq = ctx.enter_context(tc.tile_pool(name="pq", bufs=2, space="PSUM"))
    pk = ctx.enter_context(tc.tile_pool(name="pk", bufs=2, space="PSUM"))
    pv = ctx.enter_context(tc.tile_pool(name="pv", bufs=1, space="PSUM"))
    pagg = ctx.enter_context(tc.tile_pool(name="pagg", bufs=1, space="PSUM"))
    psm = ctx.enter_context(tc.tile_pool(name="psm", bufs=2, space="PSUM"))

    def ln_tile(x_sb, rows):
        """x_sb [128, W] node-major -> (xx_sb [128, W], xT chunks in psum copied to sbuf)."""
        # transpose x -> xT (2 chunks)
        xT = []
        for cche in range(2):
            ps = psm.tile([P, 2 * P], F32, tag="ps_small")
            nc.tensor.matmul(out=ps[:, 0:P], lhsT=x_sb[:, cche * P:(cche + 1) * P],
                             rhs=ident[:], is_transpose=True, start=True, stop=True)
            t = sb.tile([P, P], F32, tag=f"xT{cche}")
            nc.vector.tensor_copy(t[:], ps[:, 0:P])
            xT.append(t)
        pre = pq.tile([P, HID], F32, tag="q")
        nc.tensor.matmul(out=pre[:, 0:W], lhsT=xT[0][:], rhs=w_pre[0][:], start=True, stop=False)
        nc.tensor.matmul(out=pre[:, 0:W], lhsT=xT[1][:], rhs=w_pre[1][:], start=False, stop=False)
        nc.tensor.matmul(out=pre[:, 0:W], lhsT=ones1[:], rhs=small_rows["b_pre"][:], start=False, stop=True)
        # LayerNorm over free dim (W)
        ssum = sb.tile([P, 1], F32, tag="ssum")
        nc.vector.tensor_reduce(ssum[:], pre[:, 0:W], axis=mybir.AxisListType.X, op=mybir.AluOpType.add)
        mean = sb.tile([P, 1], F32, tag="mean")
        nc.scalar.activation(mean[:], ssum[:], mybir.ActivationFunctionType.Copy, scale=1.0 / W)
        cent = sb.tile([P, W], F32, tag="cent")
        nc.vector.tensor_scalar(out=cent[:], in0=pre[:, 0:W], scalar1=mean[:], scalar2=None,
                                op0=mybir.AluOpType.subtract)
        sq = sb.tile([P, W], F32, tag="sq")
        var = sb.tile([P, 1], F32, tag="var")
        nc.scalar.activation(sq[:], cent[:], mybir.ActivationFunctionType.Square, bias=zero1[:], accum_out=var[:])
        std = sb.tile([P, 1], F32, tag="std")
        nc.scalar.activation(std[:], var[:], mybir.ActivationFunctionType.Sqrt,
                             scale=1.0 / W, bias=eps1[:])
        rstd = sb.tile([P, 1], F32, tag="rstd")
        nc.vector.reciprocal(rstd[:], std[:])
        xx_sb = sb.tile([P, W], F32, tag="xx")
        nc.vector.tensor_scalar(out=xx_sb[:], in0=cent[:], scalar1=rstd[:], scalar2=None,
                                op0=mybir.AluOpType.mult)
        return xx_sb

    def xx_to_xxT(xx_sb, dest_tiles, dest_off):
        for cche in range(2):
            ps = psm.tile([P, 2 * P], F32, tag="ps_small")
            nc.tensor.matmul(out=ps[:, 0:P], lhsT=xx_sb[:, cche * P:(cche + 1) * P],
                             rhs=ident[:], is_transpose=True, start=True, stop=True)
            nc.vector.tensor_copy(dest_tiles[cche][:, dest_off:dest_off + P], ps[:, 0:P])

    # ---------------- Phase A: global K/V tables ----------------
    for g in range(NT_A):
        rows = min(P, N - g * P)
        x_t = sb.tile([P, W], F32, tag="x_t")
        if rows < P:
            nc.gpsimd.memset(x_t[:], 0.0)
        nc.sync.dma_start(x_t[0:rows, :], d["x"][g * P : g * P + rows, :])
        xx_sb = ln_tile(x_t, rows)
        xxT = []
        for cche in range(2):
            ps = psm.tile([P, 2 * P], F32, tag="ps_small")
            nc.tensor.matmul(out=ps[:, 0:P], lhsT=xx_sb[:, cche * P:(cche + 1) * P],
                             rhs=ident[:], is_transpose=True, start=True, stop=True)
            t = sb.tile([P, P], F32, tag=f"xxTt{cche}")
            nc.vector.tensor_copy(t[:], ps[:, 0:P])
            xxT.append(t)
        kps = pk.tile([P, HID], F32, tag="k")
        nc.tensor.matmul(out=kps[:], lhsT=xxT[0][:], rhs=w_k[0][:], start=True, stop=False)
        nc.tensor.matmul(out=kps[:], lhsT=xxT[1][:], rhs=w_k[1][:], start=False, stop=False)
        nc.tensor.matmul(out=kps[:], lhsT=ones1[:], rhs=small_rows["bk_tot"][:], start=False, stop=True)
        k_sb = sb.tile([P, HID], F32, tag="k_sb")
        nc.scalar.activation(k_sb[:], kps[:], mybir.ActivationFunctionType.Copy)
        nc.sync.dma_start(ktab[g * P : g * P + rows, :], k_sb[0:rows, :])
        vps = pv.tile([P, HID], F32, tag="v")
        nc.tensor.matmul(out=vps[:], lhsT=xxT[0][:], rhs=w_v[0][:], start=True, stop=False)
        nc.tensor.matmul(out=vps[:], lhsT=xxT[1][:], rhs=w_v[1][:], start=False, stop=False)
        nc.tensor.matmul(out=vps[:], lhsT=ones1[:], rhs=small_rows["bv_tot"][:], start=False, stop=True)
        v_sb = sb.tile([P, HID], F32, tag="v_sb")
        nc.vector.tensor_copy(v_sb[:], vps[:])
        nc.sync.dma_start(vtab[g * P : g * P + rows, :], v_sb[0:rows, :])

    # ---------------- Phase A2: local xx -> xxT_loc ----------------
    for b in range(BLOCKS):
        x_t = sb.tile([P, W], F32, tag="x_t")
        nc.sync.dma_start(x_t[:], d["xg"][b * P : (b + 1) * P, :])
        xx_sb = ln_tile(x_t, P)
        xx_to_xxT(xx_sb, xxT_loc, b * P)

    # ---------------- Phase B/C: edge blocks ----------------
    for b in range(BLOCKS):
        # Q for this block
        qblk_ps = pq.tile([P, HID], F32, tag="q")
        nc.tensor.matmul(out=qblk_ps[:], lhsT=xxT_loc[0][:, b * P:(b + 1) * P], rhs=w_q[0][:], start=True, stop=False)
        nc.tensor.matmul(out=qblk_ps[:], lhsT=xxT_loc[1][:, b * P:(b + 1) * P], rhs=w_q[1][:], start=False, stop=False)
        nc.tensor.matmul(out=qblk_ps[:], lhsT=ones1[:], rhs=small_rows["bq_tot"][:], start=False, stop=True)
        qblk = sb2.tile([P, HID], F32, tag="qblk")
        nc.scalar.activation(qblk[:], qblk_ps[:], mybir.ActivationFunctionType.Copy)

        agg_ps = pagg.tile([P, HID], F32, tag="agg")

        for t in range(t_max):
            bt = b * t_max + t
            ebase = bt * P
            # loads
            ee_t = sb.tile([P, EW], F32, tag="ee_t")
            nc.sync.dma_start(ee_t[:], d["ee_sh"][ebase : ebase + P, :])
            oh12_t = sb.tile([12, P], F32, tag="oh12_t")
            nc.sync.dma_start(oh12_t[:], d["oh12"][:, ebase : ebase + P])
            kg = sb.tile([P, HID], F32, tag="kg")
            nc.gpsimd.indirect_dma_start(
                out=kg[:], out_offset=None, in_=ktab[:, :],
                in_offset=bass.IndirectOffsetOnAxis(ap=srcw[:, bt : bt + 1], axis=0))
            vg = sb.tile([P, HID], F32, tag="vg")
            nc.gpsimd.indirect_dma_start(
                out=vg[:], out_offset=None, in_=vtab[:, :],
                in_offset=bass.IndirectOffsetOnAxis(ap=srcw[:, bt : bt + 1], axis=0))
            # eeT = 0.5*(emb12 @ oh12 + edge_embed^T)
            ps_ee = psm.tile([P, 2 * P], F32, tag="ps_small")
            nc.tensor.matmul(out=ps_ee[:, 0:P], lhsT=emb12[:], rhs=oh12_t[:], start=True, stop=False)
            nc.tensor.matmul(out=ps_ee[:, 0:P], lhsT=ee_t[:], rhs=ident[:], is_transpose=True,
                             start=False, stop=True)
            eeT = sb.tile([P, P], F32, tag="eeT")
            nc.scalar.activation(eeT[:], ps_ee[:, 0:P], mybir.ActivationFunctionType.Copy, scale=0.5)
            # one-hots: OH_en[e,n] then OH_ne = transpose
            oh_en = sb.tile([P, P], F32, tag="oh_en")
            nc.gpsimd.tensor_scalar(out=oh_en[:], in0=iota_bc[:], scalar1=dstrelf[:, bt : bt + 1],
                                    scalar2=None, op0=mybir.AluOpType.is_equal)
            ps_oh = psm.tile([P, 2 * P], F32, tag="ps_small")
            nc.tensor.matmul(out=ps_oh[:, P : 2 * P], lhsT=oh_en[:], rhs=ident[:],
                             is_transpose=True, start=True, stop=True)
            oh_ne = sb.tile([P, P], F32, tag="oh_ne")
            nc.vector.tensor_copy(oh_ne[:], ps_oh[:, P : 2 * P])
            # x_q / x_k / x_v in PSUM
            qps = pq.tile([P, HID], F32, tag="q")
            nc.tensor.matmul(out=qps[:], lhsT=eeT[:], rhs=w_r[0][:], start=True, stop=False)
            nc.tensor.matmul(out=qps[:], lhsT=oh_ne[:], rhs=qblk[:], start=False, stop=True)
            kps = pk.tile([P, HID], F32, tag="k")
            nc.tensor.matmul(out=kps[:], lhsT=eeT[:], rhs=w_r[1][:], start=True, stop=False)
            nc.tensor.matmul(out=kps[:], lhsT=ident[:], rhs=kg[:], start=False, stop=True)
            vps = pv.tile([P, HID], F32, tag="v")
            nc.tensor.matmul(out=vps[:], lhsT=eeT[:], rhs=w_r[2][:], start=True, stop=False)
            nc.tensor.matmul(out=vps[:], lhsT=ident[:], rhs=vg[:], start=False, stop=True)
            # attention
            xk = sb.tile([P, HID], F32, tag="xk")
            nc.scalar.activation(xk[:], kps[:], mybir.ActivationFunctionType.Copy)
            xv = sb.tile([P, HID], F32, tag="xv")
            nc.scalar.activation(xv[:], vps[:], mybir.ActivationFunctionType.Gelu, bias=zero1[:])
            qk = sb.tile([P, HID], F32, tag="qk")
            nc.vector.tensor_tensor(out=qk[:], in0=xk[:], in1=qps[:], op=mybir.AluOpType.mult)
            att_raw = sb.tile([P, NH], F32, tag="att_raw")
            nc.vector.tensor_reduce(att_raw[:], qk[:].rearrange("p (h d) -> p h d", h=NH),
                                    axis=mybir.AxisListType.X, op=mybir.AluOpType.add)
            att = sb.tile([P, NH], F32, tag="att")
            nc.scalar.activation(att[:], att_raw[:], mybir.ActivationFunctionType.Exp,
                                 scale=att_scale_g, bias=attb1[:])
            msg = sb.tile([P, HID], F32, tag="msg")
            nc.vector.tensor_tensor(
                out=msg[:].rearrange("p (h d) -> p h d", h=NH),
                in0=xv[:].rearrange("p (h d) -> p h d", h=NH),
                in1=att[:, :, None].to_broadcast((P, NH, HD)),
                op=mybir.AluOpType.mult)
            # segment-sum into agg
            nc.tensor.matmul(out=agg_ps[:], lhsT=oh_en[:], rhs=msg[:],
                             start=(t == 0), stop=(t == t_max - 1))

        # ---- Phase C for this block ----
        m0ps = pq.tile([P, HID], F32, tag="q")
        nc.tensor.matmul(out=m0ps[:], lhsT=xxT_loc[0][:, b * P:(b + 1) * P], rhs=w_msg0[0][:], start=True, stop=False)
        nc.tensor.matmul(out=m0ps[:], lhsT=xxT_loc[1][:, b * P:(b + 1) * P], rhs=w_msg0[1][:], start=False, stop=False)
        nc.tensor.matmul(out=m0ps[:], lhsT=ones1[:], rhs=small_rows["b_msg0"][:], start=False, stop=True)
        xx2a = sb2.tile([P, HID], F32, tag="xx2a")
        nc.scalar.activation(xx2a[:], m0ps[:], mybir.ActivationFunctionType.Gelu, bias=zero1[:])
        xx2 = sb2.tile([P, HID], F32, tag="xx2")
        nc.vector.tensor_tensor(out=xx2[:], in0=xx2a[:], in1=agg_ps[:], op=mybir.AluOpType.add)
        # transpose xx2 (4 chunks) -> lhsT for W_post
        tps = pk.tile([P, HID], F32, tag="k")
        for cche in range(4):
            nc.tensor.matmul(out=tps[:, cche * P:(cche + 1) * P], lhsT=xx2[:, cche * P:(cche + 1) * P],
                             rhs=ident[:], is_transpose=True, start=True, stop=True)
        xx2T = sb2.tile([P, HID], F32, tag="xx2T")
        nc.vector.tensor_copy(xx2T[:], tps[:])
        xg_t = sb2.tile([P, W], F32, tag="xg_t")
        nc.sync.dma_start(xg_t[:], d["xg"][b * P : (b + 1) * P, :])
        ops_ = pv.tile([P, HID], F32, tag="v")
        for cche in range(4):
            nc.tensor.matmul(out=ops_[:, 0:W], lhsT=xx2T[:, cche * P:(cche + 1) * P], rhs=w_post[cche][:],
                             start=(cche == 0), stop=False)
        nc.tensor.matmul(out=ops_[:, 0:W], lhsT=ones1[:], rhs=small_rows["b_post"][:], start=False, stop=False)
        nc.tensor.matmul(out=ops_[:, 0:W], lhsT=ident[:], rhs=xg_t[:], start=False, stop=True)
        out_sb = sb2.tile([P, W], F32, tag="out_sb")
        nc.scalar.activation(out_sb[:], ops_[:, 0:W], mybir.ActivationFunctionType.Copy)
        nc.sync.dma_start(out_d[b * P : (b + 1) * P, :], out_sb[:])

    ctx.close()


# globals threaded into _emit (set in kernel())
att_scale_g = 0.0
att_bias_g = 0.0


def kernel(**inputs):
    global att_scale_g, att_bias_g
    weights, shards, t_max, e_pad, nt, att_scale, att_bias = _host_prep(inputs)
    att_scale_g, att_bias_g = att_scale, att_bias

    nc = _build_program(t_max, e_pad, nt, att_scale, att_bias)

    in_maps = []
    for c in range(NCORES):
        m = dict(weights)
        m.update(shards[c])
        in_maps.append(m)

    res = run_bass_kernel_spmd(nc, in_maps, core_ids=list(range(NCORES)))
    global _last_result
    _last_result = res
    outs = res.results if hasattr(res, "results") else res
    full = np.zeros((N, W), np.float32)
    for c in range(NCORES):
        n0 = c * NLOC
        n1 = min(n0 + NLOC, N)
        full[n0:n1] = outs[c]["out"][0 : n1 - n0]
    return full


# revision 12
# speedup vs baseline: 1.0649x; 1.0649x over previous
"""GNN message-passing kernel for Trainium2 (8 NeuronCores, SPMD).

Strategy:
  - Sort edges by dst on host; each core owns a contiguous 1280-node range
    (10 blocks of 128 nodes) and all edges targeting it. No collectives.
  - Node-level K/V tables (xx@W_k+b etc.) computed replicated on every core,
    stored in DRAM, gathered per-edge via indirect DMA (src is uniform).
  - Q is never gathered: per 128-node block, Q_blk is computed on the fly and
    expanded to edges with a one-hot matmul (lhsT=OH[node,edge]).
  - The segment-sum (scatter-add by dst) is a one-hot matmul per tile
    accumulating into a per-block PSUM bank.
  - All biases are folded into the tables / K=1 bias-row matmuls on host.
"""

import numpy as np

import concourse.bass as bass
import concourse.tile as tile
from concourse import bacc, mybir
from concourse.bass_utils import run_bass_kernel_spmd
from concourse.masks import make_identity

# problem constants (hardcoded per contract)
N = 10000
E = 160000
W = 256
HID = 512
EW = 128
HD = 64
NH = 8
NCORES = 8
P = 128
BLOCKS = 10            # node blocks per core
NLOC = BLOCKS * P      # 1280 local node slots per core
NT_A = (N + P - 1) // P  # 79 node tiles for table build

F32 = mybir.dt.float32
I32 = mybir.dt.int32


def _host_prep(inputs):
    """Sort edges by dst, shard by owner core, pad blocks to uniform tiles."""
    x = np.ascontiguousarray(np.asarray(inputs["x"], np.float32))
    ei = np.asarray(inputs["edge_index"], np.int64)
    attr = np.asarray(inputs["edge_attr"], np.int64)
    eemb = np.ascontiguousarray(np.asarray(inputs["edge_embed"], np.float32))

    src, dst = ei[0], ei[1]
    order = np.argsort(dst, kind="stable")
    src_s = src[order].astype(np.int32)
    dst_s = dst[order].astype(np.int32)
    attr_s = attr[order].astype(np.int32)
    eemb_s = eemb[order]

    # per global 128-node block: edge ranges
    n_gblocks = NCORES * BLOCKS  # 80 slots (last ones may be empty)
    block_of_edge = dst_s // P
    starts = np.searchsorted(block_of_edge, np.arange(n_gblocks))
    ends = np.searchsorted(block_of_edge, np.arange(n_gblocks) + 1)
    counts = ends - starts
    t_max = max(1, int(np.max((counts + P - 1) // P)))
    e_pad = BLOCKS * t_max * P
    nt = BLOCKS * t_max  # edge tiles per core

    # weight prep
    init0_e = np.asarray(inputs["init0_e"], np.float32)
    init0 = np.asarray(inputs["init0"], np.float32)
    xw = np.exp(init0_e) / np.sqrt(np.sum(np.exp(init0_e)))
    s_e = float(np.exp(init0[-2]))
    s_v = float(np.exp(init0[-1]))
    att_scale = float(init0[0]) / np.sqrt(HD)
    att_bias = float(init0[1])

    emb12 = np.zeros((12, EW), np.float32)
    for t in range(4):
        tab = np.asarray(inputs[f"emb{t}"], np.float32)
        emb12[3 * t : 3 * t + 3] = tab[0:3] * xw[t]

    wr0 = np.asarray(inputs["W_r0"], np.float32) * s_e
    wr1 = np.asarray(inputs["W_r1"], np.float32) * s_e
    wr2 = np.asarray(inputs["W_r2"], np.float32) * s_v
    bq = (np.asarray(inputs["b_q"]) + np.asarray(inputs["b_r0"]) * s_e).astype(np.float32)
    bk = (np.asarray(inputs["b_k"]) + np.asarray(inputs["b_r1"]) * s_e).astype(np.float32)
    bv = (np.asarray(inputs["b_v"]) + np.asarray(inputs["b_r2"]) * s_v).astype(np.float32)

    weights = {
        "W_pre": np.ascontiguousarray(inputs["W_pre"], np.float32),
        "b_pre": np.asarray(inputs["b_pre"], np.float32).reshape(1, W),
        "W_q": np.ascontiguousarray(inputs["W_q"], np.float32),
        "W_k": np.ascontiguousarray(inputs["W_k"], np.float32),
        "W_v": np.ascontiguousarray(inputs["W_v"], np.float32),
        "W_msg0": np.ascontiguousarray(inputs["W_msg0"], np.float32),
        "b_msg0": np.asarray(inputs["b_msg0"], np.float32).reshape(1, HID),
        "W_post": np.ascontiguousarray(inputs["W_post"], np.float32),
        "b_post": np.asarray(inputs["b_post"], np.float32).reshape(1, W),
        "W_r0s": wr0, "W_r1s": wr1, "W_r2s": wr2,
        "bq_tot": bq.reshape(1, HID), "bk_tot": bk.reshape(1, HID),
        "bv_tot": bv.reshape(1, HID),
        "emb12": emb12,
        "iota_row": np.arange(P, dtype=np.float32).reshape(1, P),
        "x": x,
    }

    # per-core shard arrays
    shards = []
    for c in range(NCORES):
        ee_sh = np.zeros((e_pad, EW), np.float32)
        oh12 = np.zeros((12, e_pad), np.float32)
        srcw = np.zeros((P, nt), np.int32)
        dstrelf = np.full((P, nt), 999.0, np.float32)
        for b in range(BLOCKS):
            g = c * BLOCKS + b
            if g >= n_gblocks:
                continue
            s0, e0 = int(starts[g]), int(ends[g])
            cnt = e0 - s0
            if cnt == 0:
                continue
            base = b * t_max * P
            ee_sh[base : base + cnt] = eemb_s[s0:e0]
            a = attr_s[s0:e0]  # [cnt, 4] values in 0..2
            cols = np.arange(cnt)
            for t in range(4):
                oh12[3 * t + 0, base + cols] = (a[:, t] == 0)
                oh12[3 * t + 1, base + cols] = (a[:, t] == 1)
                oh12[3 * t + 2, base + cols] = (a[:, t] == 2)
            # wrapped [128, tile] layouts
            for ti in range((cnt + P - 1) // P):
                lo = ti * P
                hi = min(lo + P, cnt)
                col = b * t_max + ti
                srcw[0 : hi - lo, col] = src_s[s0 + lo : s0 + hi]
                dstrelf[0 : hi - lo, col] = (dst_s[s0 + lo : s0 + hi] - g * P).astype(np.float32)
        xg = np.zeros((NLOC, W), np.float32)
        n0 = c * NLOC
        n1 = min(n0 + NLOC, N)
        xg[0 : n1 - n0] = x[n0:n1]
        shards.append({"ee_sh": ee_sh, "oh12": oh12, "srcw": srcw,
                       "dstrelf": dstrelf, "xg": xg})
    return weights, shards, t_max, e_pad, nt, att_scale, att_bias


def _build_program(t_max, e_pad, nt, att_scale, att_bias):
    global att_scale_g, att_bias_g
    att_scale_g, att_bias_g = att_scale, att_bias
    nc = bacc.Bacc("TRN2", target_bir_lowering=False, debug=False,
                   num_devices=NCORES)

    # DRAM tensors
    d = {}
    d["x"] = nc.dram_tensor("x", [N, W], F32, kind="ExternalInput").ap()
    d["xg"] = nc.dram_tensor("xg", [NLOC, W], F32, kind="ExternalInput").ap()
    d["ee_sh"] = nc.dram_tensor("ee_sh", [e_pad, EW], F32, kind="ExternalInput").ap()
    d["oh12"] = nc.dram_tensor("oh12", [12, e_pad], F32, kind="ExternalInput").ap()
    d["srcw"] = nc.dram_tensor("srcw", [P, nt], I32, kind="ExternalInput").ap()
    d["dstrelf"] = nc.dram_tensor("dstrelf", [P, nt], F32, kind="ExternalInput").ap()
    for nm, shp in [("W_pre", [W, W]), ("b_pre", [1, W]),
                    ("W_q", [W, HID]), ("W_k", [W, HID]), ("W_v", [W, HID]),
                    ("W_msg0", [W, HID]), ("b_msg0", [1, HID]),
                    ("W_post", [HID, W]), ("b_post", [1, W]),
                    ("W_r0s", [EW, HID]), ("W_r1s", [EW, HID]), ("W_r2s", [EW, HID]),
                    ("bq_tot", [1, HID]), ("bk_tot", [1, HID]), ("bv_tot", [1, HID]),
                    ("emb12", [12, EW]), ("iota_row", [1, P])]:
        d[nm] = nc.dram_tensor(nm, shp, F32, kind="ExternalInput").ap()
    ktab = nc.dram_tensor("ktab", [N, HID], F32, kind="Internal").ap()
    vtab = nc.dram_tensor("vtab", [N, HID], F32, kind="Internal").ap()
    out_d = nc.dram_tensor("out", [NLOC, W], F32, kind="ExternalOutput").ap()

    with tile.TileContext(nc) as tc:
        _emit(tc, d, ktab, vtab, out_d, t_max)

    nc.compile()
    return nc


def _emit(tc, d, ktab, vtab, out_d, t_max):
    from contextlib import ExitStack

    nc = tc.nc
    ctx = ExitStack()
    wp = ctx.enter_context(tc.tile_pool(name="weights", bufs=1))

    # persistent small tensors
    ident = wp.tile([P, P], F32, tag="ident")
    make_identity(nc, ident[:])
    ones1 = wp.tile([1, P], F32, tag="ones1")
    nc.gpsimd.memset(ones1[:], 1.0)
    iota_bc = wp.tile([P, P], F32, tag="iota_bc")
    nc.sync.dma_start(iota_bc[:], d["iota_row"][0:1, :].to_broadcast((P, P)))
    zero1 = wp.tile([P, 1], F32, tag="zero1")
    nc.vector.memset(zero1[:], 0.0)
    eps1 = wp.tile([P, 1], F32, tag="eps1")
    nc.vector.memset(eps1[:], 1e-5)
    attb1 = wp.tile([P, 1], F32, tag="attb1")
    nc.vector.memset(attb1[:], att_bias_g)

    def load_w(name, parts, cols):
        ts_ = []
        for i in range(parts):
            t = wp.tile([P, cols], F32, tag=f"{name}{i}")
            nc.sync.dma_start(t[:], d[name][i * P : (i + 1) * P, :])
            ts_.append(t)
        return ts_

    w_pre = load_w("W_pre", 2, W)
    w_q = load_w("W_q", 2, HID)
    w_k = load_w("W_k", 2, HID)
    w_v = load_w("W_v", 2, HID)
    w_msg0 = load_w("W_msg0", 2, HID)
    w_post = load_w("W_post", 4, W)
    w_r = []
    for i in range(3):
        t = wp.tile([EW, HID], F32, tag=f"wr{i}")
        nc.sync.dma_start(t[:], d[f"W_r{i}s"][:, :])
        w_r.append(t)
    emb12 = wp.tile([12, EW], F32, tag="emb12")
    nc.sync.dma_start(emb12[:], d["emb12"][:, :])
    small_rows = {}
    for nm, cols in [("b_pre", W), ("bq_tot", HID), ("bk_tot", HID),
                     ("bv_tot", HID), ("b_msg0", HID), ("b_post", W)]:
        t = wp.tile([1, cols], F32, tag=nm)
        nc.sync.dma_start(t[:], d[nm][:, :])
        small_rows[nm] = t
    srcw = wp.tile([P, d["srcw"].shape[1]], I32, tag="srcw")
    nc.sync.dma_start(srcw[:], d["srcw"][:, :])
    dstrelf = wp.tile([P, d["dstrelf"].shape[1]], F32, tag="dstrelf")
    nc.sync.dma_start(dstrelf[:], d["dstrelf"][:, :])

    xxT_loc0 = wp.tile([P, NLOC], F32, tag="xxT0")
    xxT_loc1 = wp.tile([P, NLOC], F32, tag="xxT1")
    xxT_loc = [xxT_loc0, xxT_loc1]

    # pools
    sb = ctx.enter_context(tc.tile_pool(name="sb", bufs=3))
    sb2 = ctx.enter_context(tc.tile_pool(name="sb2", bufs=2))
    pq = ctx.enter_context(tc.tile_pool(name="pq", bufs=2, space="PSUM"))
    pk = ctx.enter_context(tc.tile_pool(name="pk", bufs=2, space="PSUM"))
    pv = ctx.enter_context(tc.tile_pool(name="pv", bufs=1, space="PSUM"))
    pagg = ctx.enter_context(tc.tile_pool(name="pagg", bufs=1, space="PSUM"))
    psm = ctx.enter_context(tc.tile_pool(name="psm", bufs=2, space="PSUM"))

    def ln_tile(x_sb, rows):
        """x_sb [128, W] node-major -> (xx_sb [128, W], xT chunks in psum copied to sbuf)."""
        # transpose x -> xT (2 chunks)
        xT = []
        for cche in range(2):
            ps = psm.tile([P, 2 * P], F32, tag="ps_small")
            nc.tensor.matmul(out=ps[:, 0:P], lhsT=x_sb[:, cche * P:(cche + 1) * P],
                             rhs=ident[:], is_transpose=True, start=True, stop=True)
            t = sb.tile([P, P], F32, tag=f"xT{cche}")
            nc.vector.tensor_copy(t[:], ps[:, 0:P])
            xT.append(t)
        pre = pq.tile([P, HID], F32, tag="q")
        nc.tensor.matmul(out=pre[:, 0:W], lhsT=xT[0][:], rhs=w_pre[0][:], start=True, stop=False)
        nc.tensor.matmul(out=pre[:, 0:W], lhsT=xT[1][:], rhs=w_pre[1][:], start=False, stop=False)
        nc.tensor.matmul(out=pre[:, 0:W], lhsT=ones1[:], rhs=small_rows["b_pre"][:], start=False, stop=True)
        # LayerNorm over free dim (W)
        ssum = sb.tile([P, 1], F32, tag="ssum")
        nc.vector.tensor_reduce(ssum[:], pre[:, 0:W], axis=mybir.AxisListType.X, op=mybir.AluOpType.add)
        mean = sb.tile([P, 1], F32, tag="mean")
        nc.scalar.activation(mean[:], ssum[:], mybir.ActivationFunctionType.Copy, scale=1.0 / W)
        cent = sb.tile([P, W], F32, tag="cent")
        nc.vector.tensor_scalar(out=cent[:], in0=pre[:, 0:W], scalar1=mean[:], scalar2=None,
                                op0=mybir.AluOpType.subtract)
        sq = sb.tile([P, W], F32, tag="sq")
        var = sb.tile([P, 1], F32, tag="var")
        nc.scalar.activation(sq[:], cent[:], mybir.ActivationFunctionType.Square, bias=zero1[:], accum_out=var[:])
        std = sb.tile([P, 1], F32, tag="std")
        nc.scalar.activation(std[:], var[:], mybir.ActivationFunctionType.Sqrt,
                             scale=1.0 / W, bias=eps1[:])
        rstd = sb.tile([P, 1], F32, tag="rstd")
        nc.vector.reciprocal(rstd[:], std[:])
        xx_sb = sb.tile([P, W], F32, tag="xx")
        nc.vector.tensor_scalar(out=xx_sb[:], in0=cent[:], scalar1=rstd[:], scalar2=None,
                                op0=mybir.AluOpType.mult)
        return xx_sb

    def xx_to_xxT(xx_sb, dest_tiles, dest_off):
        for cche in range(2):
            ps = psm.tile([P, 2 * P], F32, tag="ps_small")
            nc.tensor.matmul(out=ps[:, 0:P], lhsT=xx_sb[:, cche * P:(cche + 1) * P],
                             rhs=ident[:], is_transpose=True, start=True, stop=True)
            nc.vector.tensor_copy(dest_tiles[cche][:, dest_off:dest_off + P], ps[:, 0:P])

    # ---------------- Phase A: global K/V tables ----------------
    for g in range(NT_A):
        rows = min(P, N - g * P)
        x_t = sb.tile([P, W], F32, tag="x_t")
        if rows < P:
            nc.gpsimd.memset(x_t[:], 0.0)
        nc.sync.dma_start(x_t[0:rows, :], d["x"][g * P : g * P + rows, :])
        xx_sb = ln_tile(x_t, rows)
        xxT = []
        for cche in range(2):
            ps = psm.tile([P, 2 * P], F32, tag="ps_small")
            nc.tensor.matmul(out=ps[:, 0:P], lhsT=xx_sb[:, cche * P:(cche + 1) * P],
                             rhs=ident[:], is_transpose=True, start=True, stop=True)
            t = sb.tile([P, P], F32, tag=f"xxTt{cche}")
            nc.vector.tensor_copy(t[:], ps[:, 0:P])
            xxT.append(t)
        kps = pk.tile([P, HID], F32, tag="k")
        nc.tensor.matmul(out=kps[:], lhsT=xxT[0][:], rhs=w_k[0][:], start=True, stop=False)
        nc.tensor.matmul(out=kps[:], lhsT=xxT[1][:], rhs=w_k[1][:], start=False, stop=False)
        nc.tensor.matmul(out=kps[:], lhsT=ones1[:], rhs=small_rows["bk_tot"][:], start=False, stop=True)
        k_sb = sb.tile([P, HID], F32, tag="k_sb")
        nc.scalar.activation(k_sb[:], kps[:], mybir.ActivationFunctionType.Copy)
        nc.sync.dma_start(ktab[g * P : g * P + rows, :], k_sb[0:rows, :])
        vps = pv.tile([P, HID], F32, tag="v")
        nc.tensor.matmul(out=vps[:], lhsT=xxT[0][:], rhs=w_v[0][:], start=True, stop=False)
        nc.tensor.matmul(out=vps[:], lhsT=xxT[1][:], rhs=w_v[1][:], start=False, stop=False)
        nc.tensor.matmul(out=vps[:], lhsT=ones1[:], rhs=small_rows["bv_tot"][:], start=False, stop=True)
        v_sb = sb.tile([P, HID], F32, tag="v_sb")
        nc.vector.tensor_copy(v_sb[:], vps[:])
        nc.sync.dma_start(vtab[g * P : g * P + rows, :], v_sb[0:rows, :])

    # ---------------- Phase A2: local xx -> xxT_loc ----------------
    for b in range(BLOCKS):
        x_t = sb.tile([P, W], F32, tag="x_t")
        nc.sync.dma_start(x_t[:], d["xg"][b * P : (b + 1) * P, :])
        xx_sb = ln_tile(x_t, P)
        xx_to_xxT(xx_sb, xxT_loc, b * P)

    # ---------------- Phase B/C: edge blocks ----------------
    for b in range(BLOCKS):
        # Q for this block
        qblk_ps = pq.tile([P, HID], F32, tag="q")
        nc.tensor.matmul(out=qblk_ps[:], lhsT=xxT_loc[0][:, b * P:(b + 1) * P], rhs=w_q[0][:], start=True, stop=False)
        nc.tensor.matmul(out=qblk_ps[:], lhsT=xxT_loc[1][:, b * P:(b + 1) * P], rhs=w_q[1][:], start=False, stop=False)
        nc.tensor.matmul(out=qblk_ps[:], lhsT=ones1[:], rhs=small_rows["bq_tot"][:], start=False, stop=True)
        qblk = sb2.tile([P, HID], F32, tag="qblk")
        nc.scalar.activation(qblk[:], qblk_ps[:], mybir.ActivationFunctionType.Copy)

        agg_ps = pagg.tile([P, HID], F32, tag="agg")

        for t in range(t_max):
            bt = b * t_max + t
            ebase = bt * P
            # loads
            ee_t = sb.tile([P, EW], F32, tag="ee_t")
            nc.sync.dma_start(ee_t[:], d["ee_sh"][ebase : ebase + P, :])
            oh12_t = sb.tile([12, P], F32, tag="oh12_t")
            nc.sync.dma_start(oh12_t[:], d["oh12"][:, ebase : ebase + P])
            import os as _os
            kg = sb.tile([P, HID], F32, tag="kg")
            vg = sb.tile([P, HID], F32, tag="vg")
            if _os.environ.get("GATHER_STUB"):
                nc.sync.dma_start(kg[:], ktab[0:P, :])
                nc.sync.dma_start(vg[:], vtab[0:P, :])
            else:
                nc.gpsimd.indirect_dma_start(
                    out=kg[:], out_offset=None, in_=ktab[:, :],
                    in_offset=bass.IndirectOffsetOnAxis(ap=srcw[:, bt : bt + 1], axis=0))
                nc.gpsimd.indirect_dma_start(
                    out=vg[:], out_offset=None, in_=vtab[:, :],
                    in_offset=bass.IndirectOffsetOnAxis(ap=srcw[:, bt : bt + 1], axis=0))
            # eeT = 0.5*(emb12 @ oh12 + edge_embed^T)
            ps_ee = psm.tile([P, 2 * P], F32, tag="ps_small")
            nc.tensor.matmul(out=ps_ee[:, 0:P], lhsT=emb12[:], rhs=oh12_t[:], start=True, stop=False)
            nc.tensor.matmul(out=ps_ee[:, 0:P], lhsT=ee_t[:], rhs=ident[:], is_transpose=True,
                             start=False, stop=True)
            eeT = sb.tile([P, P], F32, tag="eeT")
            nc.scalar.activation(eeT[:], ps_ee[:, 0:P], mybir.ActivationFunctionType.Copy, scale=0.5)
            # one-hots: OH_en[e,n] then OH_ne = transpose
            oh_en = sb.tile([P, P], F32, tag="oh_en")
            nc.gpsimd.tensor_scalar(out=oh_en[:], in0=iota_bc[:], scalar1=dstrelf[:, bt : bt + 1],
                                    scalar2=None, op0=mybir.AluOpType.is_equal)
            ps_oh = psm.tile([P, 2 * P], F32, tag="ps_small")
            nc.tensor.matmul(out=ps_oh[:, P : 2 * P], lhsT=oh_en[:], rhs=ident[:],
                             is_transpose=True, start=True, stop=True)
            oh_ne = sb.tile([P, P], F32, tag="oh_ne")
            nc.vector.tensor_copy(oh_ne[:], ps_oh[:, P : 2 * P])
            # x_q / x_k / x_v in PSUM
            qps = pq.tile([P, HID], F32, tag="q")
            nc.tensor.matmul(out=qps[:], lhsT=eeT[:], rhs=w_r[0][:], start=True, stop=False)
            nc.tensor.matmul(out=qps[:], lhsT=oh_ne[:], rhs=qblk[:], start=False, stop=True)
            kps = pk.tile([P, HID], F32, tag="k")
            nc.tensor.matmul(out=kps[:], lhsT=eeT[:], rhs=w_r[1][:], start=True, stop=False)
            nc.tensor.matmul(out=kps[:], lhsT=ident[:], rhs=kg[:], start=False, stop=True)
            vps = pv.tile([P, HID], F32, tag="v")
            nc.tensor.matmul(out=vps[:], lhsT=eeT[:], rhs=w_r[2][:], start=True, stop=False)
            nc.tensor.matmul(out=vps[:], lhsT=ident[:], rhs=vg[:], start=False, stop=True)
            # attention
            xk = sb.tile([P, HID], F32, tag="xk")
            nc.scalar.activation(xk[:], kps[:], mybir.ActivationFunctionType.Copy)
            xv = sb.tile([P, HID], F32, tag="xv")
            nc.scalar.activation(xv[:], vps[:], mybir.ActivationFunctionType.Gelu, bias=zero1[:])
            qk = sb.tile([P, HID], F32, tag="qk")
            nc.vector.tensor_tensor(out=qk[:], in0=xk[:], in1=qps[:], op=mybir.AluOpType.mult)
            att_raw = sb.tile([P, NH], F32, tag="att_raw")
            nc.vector.tensor_reduce(att_raw[:], qk[:].rearrange("p (h d) -> p h d", h=NH),
                                    axis=mybir.AxisListType.X, op=mybir.AluOpType.add)
            att = sb.tile([P, NH], F32, tag="att")
            nc.scalar.activation(att[:], att_raw[:], mybir.ActivationFunctionType.Exp,
                                 scale=att_scale_g, bias=attb1[:])
            msg = sb.tile([P, HID], F32, tag="msg")
            nc.vector.tensor_tensor(
                out=msg[:].rearrange("p (h d) -> p h d", h=NH),
                in0=xv[:].rearrange("p (h d) -> p h d", h=NH),
                in1=att[:, :, None].to_broadcast((P, NH, HD)),
                op=mybir.AluOpType.mult)
            # segment-sum into agg
            nc.tensor.matmul(out=agg_ps[:], lhsT=oh_en[:], rhs=msg[:],
                             start=(t == 0), stop=(t == t_max - 1))

        # ---- Phase C for this block ----
        m0ps = pq.tile([P, HID], F32, tag="q")
        nc.tensor.matmul(out=m0ps[:], lhsT=xxT_loc[0][:, b * P:(b + 1) * P], rhs=w_msg0[0][:], start=True, stop=False)
        nc.tensor.matmul(out=m0ps[:], lhsT=xxT_loc[1][:, b * P:(b + 1) * P], rhs=w_msg0[1][:], start=False, stop=False)
        nc.tensor.matmul(out=m0ps[:], lhsT=ones1[:], rhs=small_rows["b_msg0"][:], start=False, stop=True)
        xx2a = sb2.tile([P, HID], F32, tag="xx2a")
        nc.scalar.activation(xx2a[:], m0ps[:], mybir.ActivationFunctionType.Gelu, bias=zero1[:])
        xx2 = sb2.tile([P, HID], F32, tag="xx2")
        nc.vector.tensor_tensor(out=xx2[:], in0=xx2a[:], in1=agg_ps[:], op=mybir.AluOpType.add)
        # transpose xx2 (4 chunks) -> lhsT for W_post
        tps = pk.tile([P, HID], F32, tag="k")
        for cche in range(4):
            nc.tensor.matmul(out=tps[:, cche * P:(cche + 1) * P], lhsT=xx2[:, cche * P:(cche + 1) * P],
                             rhs=ident[:], is_transpose=True, start=True, stop=True)
        xx2T = sb2.tile([P, HID], F32, tag="xx2T")
        nc.vector.tensor_copy(xx2T[:], tps[:])
        xg_t = sb2.tile([P, W], F32, tag="xg_t")
        nc.sync.dma_start(xg_t[:], d["xg"][b * P : (b + 1) * P, :])
        ops_ = pv.tile([P, HID], F32, tag="v")
        for cche in range(4):
            nc.tensor.matmul(out=ops_[:, 0:W], lhsT=xx2T[:, cche * P:(cche + 1) * P], rhs=w_post[cche][:],
                             start=(cche == 0), stop=False)
        nc.tensor.matmul(out=ops_[:, 0:W], lhsT=ones1[:], rhs=small_rows["b_post"][:], start=False, stop=False)
        nc.tensor.matmul(out=ops_[:, 0:W], lhsT=ident[:], rhs=xg_t[:], start=False, stop=True)
        out_sb = sb2.tile([P, W], F32, tag="out_sb")
        nc.scalar.activation(out_sb[:], ops_[:, 0:W], mybir.ActivationFunctionType.Copy)
        nc.sync.dma_start(out_d[b * P : (b + 1) * P, :], out_sb[:])

    ctx.close()


# globals threaded into _emit (set in kernel())
att_scale_g = 0.0
att_bias_g = 0.0


def kernel(**inputs):
    global att_scale_g, att_bias_g
    weights, shards, t_max, e_pad, nt, att_scale, att_bias = _host_prep(inputs)
    att_scale_g, att_bias_g = att_scale, att_bias

    nc = _build_program(t_max, e_pad, nt, att_scale, att_bias)

    in_maps = []
    for c in range(NCORES):
        m = dict(weights)
        m.update(shards[c])
        in_maps.append(m)

    res = run_bass_kernel_spmd(nc, in_maps, core_ids=list(range(NCORES)))
    global _last_result
    _last_result = res
    outs = res.results if hasattr(res, "results") else res
    full = np.zeros((N, W), np.float32)
    for c in range(NCORES):
        n0 = c * NLOC
        n1 = min(n0 + NLOC, N)
        full[n0:n1] = outs[c]["out"][0 : n1 - n0]
    return full
